# revision 1
# baseline (speedup 1.0000x reference)
"""Trainium2 Bass kernel for nn_Block_63591285784865 (attention + dense-MoE block).

Sharding: pure token-parallel across 8 NeuronCores, no collectives.
Core c handles batch b = c//2, token half = c%2 (512 query tokens).
Each core computes LN1 + K/V for the full 1024-token batch row,
attention/proj/LN2/router/all-8-experts for its own 512 tokens.

On-device layout is feature-major: activations live as [C(partitions),
tokens(free)] so matmuls chain without transposes.  The MoE (the FLOP
bulk) runs in fp8e4m3 with DoubleRow perf mode (2x128 contraction per
pass, 2x bf16 throughput); router weighting is folded into the W1
input (relu(r*x) = r*relu(x) for r>0), so each expert's contribution
comes out pre-weighted and accumulates into a persistent PSUM tile.
Attention runs with bf16 operands; softmax weights stay bf16/f32 (fp8
there costs too much accuracy).  All stats/broadcast matmuls use f32r
moving operands (4x cheaper than f32 on the PE).  All bias terms in
this problem are zero and are dropped (asserted host-side).
"""

import numpy as np
import ml_dtypes

BF16 = ml_dtypes.bfloat16
F8E4 = ml_dtypes.float8_e4m3

B, T, C = 4, 1024, 512
H, HD = 8, 64
NE, K = 8, 2
FF = 4 * C
S = 512          # tokens per core
NCORES = 8
EPS = 1e-5
NEG = -1e9

# fp8 scales
S_SEL = 16.0     # h2q = h2 * S_SEL (quantization headroom)
S_W1 = 512.0
S_HID = 32.0     # hid = relu(ph) * (r * S_HID / (S_SEL * S_W1)) = relu(ph)*r/256
S_W2 = 512.0
SEL_ROW = 1.0                         # selector rows; scales applied on-chip
INV_ACC = 1.0 / (S_HID * S_W2)   # 1/16384

_CACHE = {}


# ---------------------------------------------------------------- tile patch
def _apply_tile_patch():
    """walrus in this container rejects >2 sem waits on one instruction; the
    TileContext exit drain accumulates one wait per proc.  Split it."""
    from concourse.tile import ScopedClock, TileContext
    from concourse.tile_sem_assignment import VectorClock

    if getattr(TileContext, "_drain_patched", False):
        return

    def _drain_and_barrier_split(self, tick_clock, wait_clock):
        nc = self.nc
        gc = tick_clock.global_clock
        n = 27
        for p in range(n):
            if gc[p] <= 0:
                continue
            vals = [gc[q] if q == p else 0 for q in range(n)]
            d = nc.sync.drain()
            wait_clock.add_sem_waits(d.ins, ScopedClock({None: VectorClock(vals)}))
        nc.all_engine_barrier()
        popped = nc._tile_sem_poison_stack.pop()
        assert popped is self._sem_poison
        nc.clear_and_free_semaphores(list(self.sems.allocated().values()))
        nc.all_engine_barrier()

    TileContext._drain_and_barrier = _drain_and_barrier_split
    TileContext._drain_patched = True


def _split_sync_waits(nc, mybir, limit=2):
    """This walrus build rejects instructions carrying more than a couple of
    semaphore waits; hoist the excess onto preceding same-engine NoOps."""
    nid = [0]
    for f in nc.m.functions:
        for bb in f.blocks:
            insts = bb.instructions
            out = []
            for inst in insts:
                limit = 1
                si = inst.sync_info
                waits = list(si.on_wait) if si and si.on_wait else []
                if len(waits) > limit:
                    keep = waits[-limit:]
                    extra = waits[:-limit]
                    for i in range(0, len(extra), limit):
                        nid[0] += 1
                        nop = mybir.InstNoOp(
                            name=f"I-waitsplit-{nid[0]}",
                            engine=inst.engine,
                            ins=[], outs=[],
                            sync_info=mybir.SyncInfo(
                                on_wait=extra[i:i + limit], on_update=[]),
                        )
                        nc.register_instruction(nop, overwrite=True)
                        out.append(nop)
                    inst.sync_info = mybir.SyncInfo(
                        on_wait=keep, on_update=list(si.on_update or []))
                out.append(inst)
            bb.instructions = out


# ---------------------------------------------------------------- program
def build_program():
    import concourse.bass as bass
    import concourse.mybir as mybir
    import concourse.tile as tile

    _apply_tile_patch()

    f32 = mybir.dt.float32
    f32r = mybir.dt.float32r
    bf16 = mybir.dt.bfloat16
    f8 = mybir.dt.float8e4
    Alu = mybir.AluOpType
    Act = mybir.ActivationFunctionType
    AX = mybir.AxisListType.X
    DR = mybir.MatmulPerfMode.DoubleRow

    nc = bass.Bass()
    dp = nc.declare_dram_parameter
    # per-core inputs
    xbt_d = dp("xbt", [C, T], f32, isOutput=False)       # x[b].T, my tokens first
    kvb_d = dp("kvb", [128, 8], f32, isOutput=False)     # per-kv-chunk additive bias
    # shared inputs
    wq_d = dp("wq", [C, H * HD], bf16, isOutput=False)
    wk_d = dp("wk", [C, H * HD], bf16, isOutput=False)
    wv_d = dp("wv", [C, H * HD], bf16, isOutput=False)
    wp_d = dp("wp", [H * HD, C], f32, isOutput=False)
    wr_d = dp("wr", [C, NE], f32, isOutput=False)
    w1_d = dp("w1q", [NE, 2, 128, 16, 2, 128], f8, isOutput=False)
    w2_d = dp("w2q", [NE, 8, 128, 4, 2, 128], f8, isOutput=False)
    selc_d = dp("selc", [NE, NE * 128], f32, isOutput=False)
    out_d = dp("out", [C, S], f32, isOutput=True)
    import os
    KDEBUG = bool(os.environ.get("KDEBUG"))
    if KDEBUG:
        x2o_d = dp("x2o", [C, S], f32, isOutput=True)
        h2o_d = dp("h2o", [C, S], f32, isOutput=True)
        rto_d = dp("rto", [NE, S], mybir.dt.float32r, isOutput=True)

    from concourse.masks import make_identity

    def recip_fast(out_ap, in_ap):
        # DVE reciprocal: scalar-engine Act.Reciprocal would thrash the
        # activation table against the Exp-heavy attention loop
        with nc.allow_low_precision(reason="f32r bcast row"):
            nc.vector.reciprocal(out_ap, in_ap)

    with tile.TileContext(nc) as tc:
        with tc.tile_pool(name="persist", bufs=1) as pp:
            # ---------------- persistent constants
            ones128 = pp.tile([128, 128], f32, name="ones128", tag="ones128")
            nc.vector.memset(ones128[:], 1.0)
            ones128r = pp.tile([128, 128], f32r, name="ones128r", tag="ones128r")
            nc.vector.tensor_copy(ones128r[:], ones128[:])
            ones128b = pp.tile([128, 128], bf16, name="ones128b", tag="ones128b")
            nc.vector.tensor_copy(ones128b[:], ones128[:])
            ident = pp.tile([128, 128], f32, name="ident", tag="ident")
            make_identity(nc, ident[:])
            eps_t = pp.tile([1, 1], f32, name="eps_t", tag="eps_t")
            nc.vector.memset(eps_t[:], EPS)
            # router one-hot selector rows (x S_SEL)
            selc32 = pp.tile([32, NE * 128], f32, name="selc32", tag="selc32")
            nc.sync.dma_start(selc32[0:NE, :], selc_d[:])
            selc32r = pp.tile([32, NE * 128], f32r, name="selc32r", tag="selc32r")
            nc.vector.tensor_copy(selc32r[0:NE, :], selc32[0:NE, :])
            rT32 = pp.tile([32, S], f32r, name="rT32", tag="rT32")
            x2T = [pp.tile([128, S], f32, name=f"x2T{ct}", tag=f"x2T{ct}")
                   for ct in range(4)]
            f8_ = mybir.dt.float8e4
            h2q = [pp.tile([128, 2, S], f8_, name=f"h2q{cp}", tag=f"h2q{cp}")
                   for cp in range(2)]
            h2f = [pp.tile([128, S], f32, name=f"h2f{ci}", tag=f"h2f{ci}")
                   for ci in range(4)]

            ones_col_f = ones128[:, 0:1]          # f32 stationary (f32 moving)
            ones_col_r = ones128r[:, 0:1]         # f32r stationary
            ones_row_r = ones128r[0:1, :]
            ones_row_m = ones128r[32:33, :]

            w1p_cm = tc.tile_pool(name="w1pool", bufs=2)
            w2p_cm = tc.tile_pool(name="w2pool", bufs=2)
            w1p = w1p_cm.__enter__()
            w2p = w2p_cm.__enter__()
            w1ts, w2ts = {}, {}

            def make_expert_tiles(e, eng):
                w1ts[e] = [w1p.tile([128, 16, 2, 128], f8, name=f"w1t{cp}",
                                    tag=f"w1_{cp}") for cp in range(2)]
                for cp in range(2):
                    eng.dma_start(w1ts[e][cp][:], w1_d[e, cp])
                w2ts[e] = [w2p.tile([128, 4, 2, 128], f8, name=f"w2t{fp}",
                                    tag=f"w2_{fp}") for fp in range(8)]
                for fp in range(8):
                    eng.dma_start(w2ts[e][fp][:], w2_d[e, fp])

            with tc.tile_pool(name="attn_era", bufs=1) as ae:
                # ---------------- attention-era constants
                kvb = ae.tile([128, 8], f32, name="kvb", tag="kvb")
                # one shared triangular mask for the diagonal 128-block
                mdiag = ae.tile([128, 128], f32, name="mdiag", tag="mdiag")
                nc.gpsimd.memset(mdiag[:], 0.0)
                nc.gpsimd.affine_select(
                    out=mdiag[:], in_=mdiag[:],
                    compare_op=Alu.is_ge, fill=NEG,
                    base=0, channel_multiplier=-1,
                    pattern=[[1, 128]],
                )
                wq = [ae.tile([128, H * HD], bf16, name=f"wq{ci}", tag=f"wq{ci}")
                      for ci in range(4)]
                wk = [ae.tile([128, H * HD], bf16, name=f"wk{ci}", tag=f"wk{ci}")
                      for ci in range(4)]
                wv = [ae.tile([128, H * HD], bf16, name=f"wv{ci}", tag=f"wv{ci}")
                      for ci in range(4)]
                wp = [ae.tile([128, C], f32r, name=f"wp{ci}", tag=f"wp{ci}")
                      for ci in range(4)]
                wr = [ae.tile([128, NE], f32, name=f"wr{ci}", tag=f"wr{ci}")
                      for ci in range(4)]
                hT = [ae.tile([128, T], bf16, name=f"hT{ci}", tag=f"hT{ci}")
                      for ci in range(4)]
                qTs = [ae.tile([128, S], bf16, name=f"qT{ft}", tag=f"qT{ft}")
                       for ft in range(4)]
                kTs = [ae.tile([128, T], bf16, name=f"kT{ft}", tag=f"kT{ft}")
                       for ft in range(4)]
                # V with a ones column appended per head: [128, 8 heads, 64+1]
                vSa = [ae.tile([128, H, HD + 1], bf16, name=f"v{j}", tag=f"v{j}")
                       for j in range(8)]
                attnT2 = [ae.tile([128, S], f32r, name=f"attnT{ft}",
                                  tag=f"attnT{ft}") for ft in range(4)]
                # LN row scratch (f32), rows at partition starts 0/32/64/96
                rows1 = ae.tile([128, 2 * T], f32, name="rows1", tag="rows1")
                rows2 = ae.tile([128, 2 * S], f32, name="rows2", tag="rows2")
                # broadcast sources (f32r): rstd at [0:1], mu*rstd at [32:33]
                bsrc1 = [ae.tile([33, 512], f32r, name=f"bsrc1_{th}",
                                 tag=f"bsrc1_{th}") for th in range(2)]
                bsrc2 = ae.tile([33, S], f32r, name="bsrc2", tag="bsrc2")

                # ---------------- LN1 (pure standardize; gain folded into W)
                with (
                    tc.tile_pool(name="ln1sb", bufs=1) as lsb,
                    tc.tile_pool(name="xbt_era", bufs=1) as xe,
                    tc.tile_pool(name="ln1ps", bufs=1, space="PSUM") as lps,
                ):
                    xbT = [xe.tile([128, T], f32, name=f"xbT{ci}", tag=f"xbT{ci}")
                           for ci in range(4)]
                    for th in range(2):
                        for ci in range(4):
                            nc.sync.dma_start(
                                xbT[ci][:, 512 * th:512 * (th + 1)],
                                xbt_d[128 * ci:128 * (ci + 1),
                                      512 * th:512 * (th + 1)])
                    nc.sync.dma_start(kvb[:], kvb_d[:])
                    for ci in range(4):
                        cs = slice(128 * ci, 128 * (ci + 1))
                        nc.sync.dma_start(wq[ci][:], wq_d[cs, :])
                        nc.sync.dma_start(wk[ci][:], wk_d[cs, :])
                        nc.sync.dma_start(wv[ci][:], wv_d[cs, :])
                        nc.sync.dma_start(wr[ci][:], wr_d[cs, :])
                    with tc.tile_pool(name="wstage", bufs=2) as wst:
                        for ci in range(4):
                            cs = slice(128 * ci, 128 * (ci + 1))
                            stg = wst.tile([128, C], f32, name="stg", tag="stg")
                            nc.sync.dma_start(stg[:], wp_d[cs, :])
                            nc.vector.tensor_copy(wp[ci][:], stg[:])
                    make_expert_tiles(0, nc.sync)
                    make_expert_tiles(1, nc.sync)

                    def scalar_recip(out_ap, in_ap):
                        eng = nc.scalar
                        ins_ = [eng.lower_ap(in_ap)]
                        for v in (0.0, 1.0, 0.0):
                            ins_.append(mybir.ImmediateValue(
                                dtype=mybir.dt.float32, value=v))
                        eng.add_instruction(mybir.InstActivation(
                            name=nc.get_next_instruction_name(),
                            func=Act.Reciprocal,
                            ins=ins_, outs=[eng.lower_ap(out_ap)]))

                    psums, psqs = [], []
                    for th in range(2):
                        ts_ = slice(512 * th, 512 * (th + 1))
                        psum = lps.tile([1, 512], f32, name="psum", tag="s",
                                        bufs=2)
                        psq = lps.tile([1, 512], f32, name="psq", tag="sq",
                                       bufs=2)
                        psums.append(psum)
                        psqs.append(psq)
                        for ci in range(4):
                            nc.tensor.matmul(psum[:], ones_col_f,
                                             xbT[ci][:, ts_],
                                             start=(ci == 0), stop=(ci == 3))
                        for ci in range(4):
                            xq_t = lsb.tile([128, 512], f32r, name="xq_t",
                                            tag="xsq", bufs=2)
                            nc.scalar.square(xq_t[:], xbT[ci][:, ts_])
                            nc.tensor.matmul(psq[:], ones_col_r, xq_t[:],
                                             start=(ci == 0), stop=(ci == 3))

                    def ln1_rows(th):
                        ts_ = slice(512 * th, 512 * (th + 1))
                        ts2 = slice(T + 512 * th, T + 512 * (th + 1))
                        mu = rows1[64:65, ts_]
                        musq = rows1[96:97, ts_]
                        var = rows1[0:1, ts2]
                        rstd = rows1[0:1, ts_]
                        sd = rows1[32:33, ts2]
                        psum, psq = psums[th], psqs[th]
                        nc.vector.tensor_scalar_mul(mu, psum[:], 1.0 / C)
                        nc.vector.tensor_tensor(musq, mu, mu, Alu.mult)
                        nc.vector.scalar_tensor_tensor(var, psq[:], 1.0 / C,
                                                       musq, Alu.mult,
                                                       Alu.subtract)
                        nc.scalar.activation(sd, var, Act.Sqrt, bias=eps_t[:])
                        scalar_recip(rstd, sd)
                        nc.scalar.copy(bsrc1[th][0:1, :], rstd)
                        nc.vector.scalar_tensor_tensor(bsrc1[th][32:33, :],
                                                       psum[:], 1.0 / C, rstd,
                                                       Alu.mult, Alu.mult)

                    def ln1_bcast_h(th):
                        ts_ = slice(512 * th, 512 * (th + 1))
                        prs = lps.tile([128, 512], f32, name="prs", tag="prs",
                                       bufs=1)
                        nc.tensor.matmul(prs[:], ones_row_r, bsrc1[th][0:1, :],
                                         start=True, stop=True)
                        pms = lps.tile([128, 512], f32, name="pms", tag="pms",
                                       bufs=1)
                        nc.tensor.matmul(pms[:], ones_row_m, bsrc1[th][32:33, :],
                                         start=True, stop=True)
                        for ci in range(4):
                            tmp = lsb.tile([128, 512], f32, name="tmp", tag="nrm")
                            nc.vector.tensor_tensor(tmp[:], xbT[ci][:, ts_],
                                                    prs[:], Alu.mult)
                            nc.vector.tensor_tensor(hT[ci][:, ts_], tmp[:],
                                                    pms[:], Alu.subtract)

                    def qkv_a():
                        for j in range(8):
                            nc.gpsimd.memset(vSa[j][:], 1.0)
                        for ft in range(4):
                            fs = slice(128 * ft, 128 * (ft + 1))
                            pq = lps.tile([128, S], f32, name="pq", tag="qkv",
                                          bufs=2)
                            for ci in range(4):
                                nc.tensor.matmul(pq[:], wq[ci][:, fs],
                                                 hT[ci][:, 0:S],
                                                 start=(ci == 0), stop=(ci == 3))
                            nc.scalar.copy(qTs[ft][:], pq[:])
                            pk = lps.tile([128, 512], f32, name="pk", tag="qkv",
                                          bufs=2)
                            for ci in range(4):
                                nc.tensor.matmul(pk[:], wk[ci][:, fs],
                                                 hT[ci][:, 0:512],
                                                 start=(ci == 0), stop=(ci == 3))
                            nc.scalar.copy(kTs[ft][:, 0:512], pk[:])
                        for j in range(4):
                            js = slice(128 * j, 128 * (j + 1))
                            pv = lps.tile([128, H * HD], f32, name="pv",
                                          tag="qkv", bufs=2)
                            for ci in range(4):
                                nc.tensor.matmul(pv[:], hT[ci][:, js], wv[ci][:],
                                                 start=(ci == 0), stop=(ci == 3))
                            nc.scalar.copy(
                                vSa[j][:, :, 0:HD],
                                pv[:].rearrange("p (h d) -> p h d", h=H))

                    def qkv_b():
                        for ft in range(4):
                            fs = slice(128 * ft, 128 * (ft + 1))
                            pk = lps.tile([128, 512], f32, name="pk", tag="qkv",
                                          bufs=2)
                            for ci in range(4):
                                nc.tensor.matmul(pk[:], wk[ci][:, fs],
                                                 hT[ci][:, 512:1024],
                                                 start=(ci == 0), stop=(ci == 3))
                            nc.scalar.copy(kTs[ft][:, 512:1024], pk[:])
                        for j in range(4, 8):
                            js = slice(128 * j, 128 * (j + 1))
                            pv = lps.tile([128, H * HD], f32, name="pv",
                                          tag="qkv", bufs=2)
                            for ci in range(4):
                                nc.tensor.matmul(pv[:], hT[ci][:, js], wv[ci][:],
                                                 start=(ci == 0), stop=(ci == 3))
                            nc.scalar.copy(
                                vSa[j][:, :, 0:HD],
                                pv[:].rearrange("p (h d) -> p h d", h=H))

                    ln1_rows(0)
                    ln1_bcast_h(0)
                    ln1_rows(1)
                    qkv_a()
                    ln1_bcast_h(1)
                    qkv_b()

                xq_r = [ae.tile([128, S], f32, name=f"xqr{ct}", tag=f"xqr{ct}")
                        for ct in range(4)]
                for ct in range(4):
                    nc.sync.dma_start(xq_r[ct][:],
                                      xbt_d[128 * ct:128 * (ct + 1), 0:S])

                # ---------------- attention; normalize tails pipelined
                # into the next head-pair's j-loop so the PE never stalls
                with (
                    tc.tile_pool(name="attnps", bufs=2, space="PSUM") as aps,
                    tc.tile_pool(name="attnsb", bufs=3) as asb,
                ):
                    tails1, tails2 = [], []

                    def make_tail(ft, sub, pav):
                        rd = asb.tile([1, S], bf16, name="rd", tag="rd",
                                      bufs=4)

                        def stage1():
                            with nc.allow_low_precision(reason="bf16 recip"):
                                nc.vector.reciprocal(rd[:], pav[64:65, :])

                        def stage2():
                            rs = slice(64 * sub, 64 * (sub + 1))
                            prb = aps.tile([64, S], f32, name="prb", tag="prb",
                                           bufs=1)
                            nc.tensor.matmul(prb[:], ones128b[0:1, 0:64],
                                             rd[:], start=True, stop=True)
                            rb = asb.tile([64, S], f32, name="rb", tag="rb")
                            nc.vector.tensor_copy(rb[:], prb[:])
                            nc.vector.tensor_tensor(attnT2[ft][rs, :],
                                                    pav[0:64, :],
                                                    rb[:], Alu.mult)
                        return stage1, stage2

                    for hp in range(4):
                        ft = hp
                        pav = [aps.tile([65, S], f32, name=f"pav{sub}",
                                        tag=f"pav{sub}", bufs=2) for sub in range(2)]
                        for jx, j in enumerate((0, 4, 1, 5, 2, 6, 3, 7)):
                            q0 = 128 * j if j < 4 else 0
                            pt2 = []
                            for sub in range(2):
                                rs = slice(64 * sub, 64 * (sub + 1))
                                ps = aps.tile([128, S], f32, name="ps", tag="ps",
                                              bufs=3)
                                nc.tensor.matmul(ps[:, q0:S],
                                                 kTs[ft][rs, 128 * j:128 * (j + 1)],
                                                 qTs[ft][rs, q0:S],
                                                 start=True, stop=True)
                                pt = asb.tile([128, S], bf16, name="pt", tag="pt",
                                              bufs=6)
                                if j < 4:
                                    tmpm = asb.tile([128, 128], f32, name="tmpm",
                                                    tag="ptmp")
                                    nc.vector.tensor_tensor(tmpm[:],
                                                            ps[:, q0:q0 + 128],
                                                            mdiag[:], Alu.add)
                                    nc.scalar.activation(pt[:, q0:q0 + 128],
                                                         tmpm[:], Act.Exp,
                                                         bias=kvb[:, j:j + 1],
                                                         scale=1.0)
                                    if q0 + 128 < S:
                                        nc.scalar.activation(pt[:, q0 + 128:S],
                                                             ps[:, q0 + 128:S],
                                                             Act.Exp,
                                                             bias=kvb[:, j:j + 1],
                                                             scale=1.0)
                                else:
                                    nc.scalar.activation(pt[:], ps[:], Act.Exp,
                                                         bias=kvb[:, j:j + 1],
                                                         scale=1.0)
                                pt2.append(pt)
                            for sub in range(2):
                                h = 2 * hp + sub
                                nc.tensor.matmul(pav[sub][:, q0:S],
                                                 vSa[j][:, h:h + 1, :],
                                                 pt2[sub][:, q0:S],
                                                 start=(jx == 0), stop=(jx == 7),
                                                 skip_group_check=True)
                            if jx in (0, 1) and tails1:
                                tails1.pop(0)()
                            if jx in (4, 5) and tails2:
                                tails2.pop(0)()
                        for sub in range(2):
                            s1, s2 = make_tail(ft, sub, pav[sub])
                            tails1.append(s1)
                            tails2.append(s2)
                    for t in tails1:
                        t()
                    for t in tails2:
                        t()

                # ------------ proj + residual -> x2T, LN2 stats interleaved
                with (
                    tc.tile_pool(name="projps", bufs=4, space="PSUM") as pps,
                    tc.tile_pool(name="ln2sb", bufs=1) as lsb2,
                    tc.tile_pool(name="ln2ps", bufs=1, space="PSUM") as lps2,
                ):
                    psum = lps2.tile([1, S], f32, name="psum2", tag="s", bufs=1)
                    psq = lps2.tile([1, S], f32, name="psq2", tag="sq", bufs=1)
                    for ct in range(4):
                        cs = slice(128 * ct, 128 * (ct + 1))
                        px = pps.tile([128, S], f32, name="px", tag="px")
                        for hd in range(4):
                            nc.tensor.matmul(px[:], wp[hd][:, cs],
                                             attnT2[hd][:],
                                             start=(hd == 0), stop=(hd == 3))
                        nc.vector.tensor_tensor(x2T[ct][:], px[:], xq_r[ct][:],
                                                Alu.add)
                        nc.tensor.matmul(psum[:], ones_col_f, x2T[ct][:],
                                         start=(ct == 0), stop=(ct == 3),
                                         skip_group_check=True)
                        xq_t = lsb2.tile([128, S], f32r, name="xq_t2",
                                         tag="xsq2", bufs=2)
                        nc.scalar.square(xq_t[:], x2T[ct][:])
                        nc.tensor.matmul(psq[:], ones_col_r, xq_t[:],
                                         start=(ct == 0), stop=(ct == 3),
                                         skip_group_check=True)

                    # LN2 rows -> h2q (fp8, x S_SEL) directly
                    mu = rows2[64:65, 0:S]
                    musq = rows2[96:97, 0:S]
                    var = rows2[0:1, S:2 * S]
                    rstd = rows2[0:1, 0:S]
                    sd2 = rows2[32:33, S:2 * S]
                    nc.vector.tensor_scalar_mul(mu, psum[:], 1.0 / C)
                    nc.vector.tensor_tensor(musq, mu, mu, Alu.mult)
                    nc.vector.scalar_tensor_tensor(var, psq[:], 1.0 / C, musq,
                                                   Alu.mult, Alu.subtract)
                    nc.scalar.activation(sd2, var, Act.Sqrt, bias=eps_t[:])
                    scalar_recip(rstd, sd2)
                    # bsrc2 rows: rstd*S_SEL at [0:1], mu*rstd*S_SEL at [32:33]
                    nc.scalar.mul(bsrc2[0:1, :], rstd, S_SEL)
                    nc.vector.scalar_tensor_tensor(bsrc2[32:33, :], psum[:],
                                                   S_SEL / C, rstd,
                                                   Alu.mult, Alu.mult)
                    prs = lps2.tile([128, S], f32, name="prs2", tag="prs", bufs=1)
                    nc.tensor.matmul(prs[:], ones_row_r, bsrc2[0:1, :],
                                     start=True, stop=True)
                    pms = lps2.tile([128, S], f32, name="pms2", tag="pms", bufs=1)
                    nc.tensor.matmul(pms[:], ones_row_m, bsrc2[32:33, :],
                                     start=True, stop=True)
                    for ci in range(4):
                        tmp = lsb2.tile([128, S], f32, name="tmp3", tag="nrm2")
                        nc.vector.tensor_tensor(tmp[:], x2T[ci][:], prs[:],
                                                Alu.mult)
                        nc.vector.tensor_tensor(h2q[ci // 2][:, ci % 2, :],
                                                tmp[:], pms[:], Alu.subtract)
                        nc.vector.tensor_tensor(h2f[ci][:], tmp[:], pms[:],
                                                Alu.subtract)

                # ---------------- router (batched over all 4 token chunks)
                with (
                    tc.tile_pool(name="rps", bufs=1, space="PSUM") as rps,
                    tc.tile_pool(name="rsb", bufs=1) as rsb,
                ):
                    pscs = [rps.tile([128, NE], f32, name=f"psc{qc}",
                                     tag=f"psc{qc}") for qc in range(4)]
                    for qc in range(4):
                        qs = slice(128 * qc, 128 * (qc + 1))
                        for ci in range(4):
                            nc.tensor.matmul(pscs[qc][:], h2f[ci][:, qs],
                                             wr[ci][:],
                                             start=(ci == 0), stop=(ci == 3))
                    rrall = rsb.tile([128, 128], f32, name="rrall", tag="rrall")
                    nc.vector.memset(rrall[:], 0.0)
                    # batched top-2 softmax chain over all 4 token chunks:
                    # copy the 4 psc tiles into one [128, 4, 8] SBUF tensor,
                    # reduce along the last dim, broadcast via stride-0 APs
                    scb = rsb.tile([128, 4, NE], f32, name="scb", tag="scb")
                    for qc in range(4):
                        # h2f carries the S_SEL prescale; undo it for softmax
                        nc.scalar.mul(scb[:, qc, :], pscs[qc][:], 1.0 / S_SEL)
                    sc3 = scb[:]
                    m1 = rsb.tile([128, 4, 1], f32, name="m1", tag="m1")
                    nc.vector.reduce_max(m1[:], sc3, axis=AX)
                    m1b = m1[:].broadcast_to([128, 4, NE])
                    eq = rsb.tile([128, 4, NE], f32, name="eq", tag="eq")
                    nc.vector.tensor_tensor(eq[:], sc3, m1b, Alu.is_equal)
                    sm = rsb.tile([128, 4, NE], f32, name="sm", tag="sm")
                    nc.vector.scalar_tensor_tensor(sm[:], eq[:], NEG, sc3,
                                                   Alu.mult, Alu.add)
                    m2 = rsb.tile([128, 4, 1], f32, name="m2", tag="m2")
                    nc.vector.reduce_max(m2[:], sm[:], axis=AX)
                    m2b = m2[:].broadcast_to([128, 4, NE])
                    ge = rsb.tile([128, 4, NE], f32, name="ge", tag="ge")
                    nc.vector.tensor_tensor(ge[:], sc3, m2b, Alu.is_ge)
                    msk = rsb.tile([128, 4, NE], f32, name="msk", tag="msk")
                    nc.vector.tensor_tensor(msk[:], sc3, ge[:], Alu.mult)
                    ex = rsb.tile([128, 4, NE], f32, name="ex", tag="ex")
                    nc.scalar.activation(ex[:], msk[:], Act.Exp)
                    dsum = rsb.tile([128, 4, 1], f32, name="dsum", tag="dsum")
                    nc.vector.reduce_sum(dsum[:], ex[:], axis=AX)
                    rec = rsb.tile([128, 4, 1], f32, name="rec", tag="rec")
                    nc.vector.reciprocal(rec[:], dsum[:])
                    recb = rec[:].broadcast_to([128, 4, NE])
                    for qc in range(4):
                        es = slice(32 * qc, 32 * qc + NE)
                        nc.vector.tensor_tensor(
                            rrall[:, es],
                            ex[:, qc, :], rec[:, qc, :].broadcast_to([128, 1, NE]),
                            Alu.mult)
                    ptr = rps.tile([128, 128], f32, name="ptr", tag="ptr")
                    nc.tensor.transpose(ptr[:], rrall[:], ident[:])
                    for qc in range(4):
                        qs = slice(128 * qc, 128 * (qc + 1))
                        nc.scalar.copy(rT32[0:NE, qs],
                                       ptr[32 * qc:32 * qc + NE, :])

            if KDEBUG:
                for ct in range(4):
                    nc.sync.dma_start(x2o_d[128 * ct:128 * (ct + 1), :],
                                      x2T[ct][:])
                    nc.sync.dma_start(h2o_d[128 * ct:128 * (ct + 1), :],
                                      h2f[ct][:])
                nc.sync.dma_start(rto_d[:], rT32[0:NE, :])

            # ---------------- MoE: fp8 DoubleRow, PSUM-resident accumulator.
            # Router weight is applied at the relu stage:
            # hid = max(ph,0)*prbs with prbs = r_e/256 broadcast, so W1 runs
            # on the shared h2q tiles and expert 0 can start while the
            # router chain is still finishing.
            with (
                tc.tile_pool(name="hidpool", bufs=4) as hpool,
                tc.tile_pool(name="accps", bufs=1, space="PSUM") as accp,
                tc.tile_pool(name="moeps", bufs=2, space="PSUM") as mps,
                tc.tile_pool(name="moesb", bufs=2) as msb,
            ):
                pacc = [accp.tile([128, S], f32, name=f"pacc{ct}", tag=f"acc{ct}")
                        for ct in range(4)]
                # explicit zero; interleaved region start=True on a bank with
                # an open accumulation group clobbers the other region
                for ct in range(4):
                    nc.vector.memset(pacc[ct][:], 0.0)
                for e in range(2, NE):
                    make_expert_tiles(e, nc.sync)

                def issue_prbs(e):
                    prbe = mps.tile([128, S], f32, name="prbe", tag="prbe",
                                    bufs=1)
                    nc.tensor.matmul(prbe[:],
                                     selc32r[0:NE, 128 * e:128 * (e + 1)],
                                     rT32[0:NE, :], start=True, stop=True)
                    # r/256 for the DVE relu path, plain r for gpsimd path
                    prbs = msb.tile([128, S], f32, name="prbs", tag="prbs")
                    nc.scalar.mul(prbs[:], prbe[:], 1.0 / 256.0)
                    prbsr = msb.tile([128, S], f32, name="prbsr", tag="prbsr")
                    nc.scalar.copy(prbsr[:], prbe[:])
                    return prbs, prbsr

                prbs_cur = issue_prbs(0)
                prbs_next = None
                pend = []

                def drain(n):
                    for _ in range(min(n, len(pend))):
                        pend.pop(0)()

                n_w2 = [0]
                ct_count = [0] * 4

                def emit_out(ct):
                    ot = msb.tile([128, S], f32, name="ot", tag="ot")
                    nc.vector.scalar_tensor_tensor(ot[:], pacc[ct][:], INV_ACC,
                                                   x2T[ct][:], Alu.mult, Alu.add)
                    nc.sync.dma_start(out_d[128 * ct:128 * (ct + 1), :], ot[:])

                def make_w2(e, fp, hid_t):
                    w2t = w2ts[e][fp]

                    def mk(ct):
                        def go():
                            n_w2[0] += 1
                            ct_count[ct] += 1
                            last = ct_count[ct] == NE * 8
                            nc.tensor.matmul(
                                pacc[ct][:], w2t[:, ct, :, :],
                                hid_t[:],
                                start=False, stop=last,
                                perf_mode=DR, skip_group_check=True)
                            if last:
                                emit_out(ct)
                        return go
                    return [mk(ct) for ct in range(4)]

                for e in range(NE):
                    hid_t = None
                    for ff in range(16):
                        fp, jj = divmod(ff, 2)
                        if jj == 0:
                            hid_t = hpool.tile([128, 2, S], f8, name="hid",
                                               tag="hid")
                        if ff == 8 and e + 1 < NE:
                            prbs_next = issue_prbs(e + 1)
                        ph = mps.tile([128, S], f32, name="ph", tag="ph",
                                      bufs=3)
                        for cp in range(2):
                            nc.tensor.matmul(
                                ph[:], w1ts[e][cp][:, ff, :, :],
                                h2q[cp][:],
                                start=(cp == 0), stop=(cp == 1),
                                perf_mode=DR)
                        if ff % 2 == 0:
                            rtmp = msb.tile([128, S], bf16, name="rtmp",
                                            tag="rtmp", bufs=3)
                            nc.scalar.activation(rtmp[:], ph[:], Act.Relu,
                                                 scale=1.0 / 256.0)
                            nc.vector.tensor_tensor(hid_t[:, jj, :], rtmp[:],
                                                    prbs_cur[1][:], Alu.mult)
                        else:
                            nc.vector.scalar_tensor_tensor(hid_t[:, jj, :],
                                                           ph[:], 0.0,
                                                           prbs_cur[0][:],
                                                           Alu.max, Alu.mult)
                        if jj == 1:
                            pend.extend(make_w2(e, fp, hid_t))
                        drain(2)
                    prbs_cur = prbs_next
                drain(len(pend))

            w2p_cm.__exit__(None, None, None)
            w1p_cm.__exit__(None, None, None)

    _split_sync_waits(nc, mybir)
    return nc


# ---------------------------------------------------------------- host side
def prep_inputs(inputs):
    """Returns list of 8 per-core input dicts."""
    x = np.asarray(inputs["x"], np.float32)
    ln1_g = np.asarray(inputs["ln1_g"], np.float32)
    ln1_b = np.asarray(inputs["ln1_b"], np.float32)
    ln2_g = np.asarray(inputs["ln2_g"], np.float32)
    ln2_b = np.asarray(inputs["ln2_b"], np.float32)
    Wq = np.asarray(inputs["Wq"], np.float32)
    Wk = np.asarray(inputs["Wk"], np.float32)
    Wv = np.asarray(inputs["Wv"], np.float32)
    Wp = np.asarray(inputs["Wp"], np.float32)
    bp = np.asarray(inputs["bp"], np.float32)
    Wr = np.asarray(inputs["Wr"], np.float32)
    br = np.asarray(inputs["br"], np.float32)
    W1 = np.asarray(inputs["W1"], np.float32)
    b1 = np.asarray(inputs["b1"], np.float32)
    W2 = np.asarray(inputs["W2"], np.float32)
    b2 = np.asarray(inputs["b2"], np.float32)

    # all bias terms in this problem are zero; the kernel drops them
    for nm, v in (("ln1_b", ln1_b), ("ln2_b", ln2_b), ("bp", bp),
                  ("br", br), ("b1", b1), ("b2", b2)):
        assert np.abs(v).max() == 0.0, f"{nm} nonzero; kernel assumes zero"

    WqT2 = Wq.transpose(1, 0, 2).reshape(C, H * HD)
    WkT2 = Wk.transpose(1, 0, 2).reshape(C, H * HD)
    WvT2 = Wv.transpose(1, 0, 2).reshape(C, H * HD)
    isq = np.float32(1.0 / np.sqrt(HD))

    wq = (ln1_g[:, None] * WqT2 * isq).astype(BF16)
    wk = (ln1_g[:, None] * WkT2).astype(BF16)
    wv = (ln1_g[:, None] * WvT2).astype(BF16)
    wr = (ln2_g[:, None] * Wr).astype(np.float32)
    w1g = ln2_g[None, :, None] * W1          # [NE, C, FF]
    # DoubleRow layouts: contraction pairs (i in {0,1}) packed per partition
    # chunk-contiguous stationary layout: [NE, grp, 128p, chunk, i, 128m]
    # with the DoubleRowSwInterleave byte order inside each 256B chunk:
    # stream byte 2k+i = logical (pair element i, column 127-k)
    j = np.arange(256)
    swi_src_i = j % 2                 # pair element
    swi_src_c = 127 - j // 2          # logical column (reversed)
    w1c = np.ascontiguousarray(
        (w1g * S_W1).reshape(NE, 2, 2, 128, 16, 128)
        .transpose(0, 1, 3, 4, 2, 5)
    )                                  # [NE, cp, p, ffc, i, m]
    w2c = np.ascontiguousarray(
        (W2 * S_W2).reshape(NE, 8, 2, 128, 4, 128)
        .transpose(0, 1, 3, 4, 2, 5)
    )                                  # [NE, fp, p, ctc, i, m]
    w1s = w1c.reshape(NE, 2, 128, 16, 256)
    w2s = w2c.reshape(NE, 8, 128, 4, 256)
    w1q = np.ascontiguousarray(w1s).astype(F8E4)   # [NE, 2, 128, 16, 256]
    w2q = np.ascontiguousarray(w2s).astype(F8E4)   # [NE, 8, 128, 4, 256]

    selc = np.zeros((NE, NE * 128), np.float32)
    for e in range(NE):
        selc[e, 128 * e:128 * (e + 1)] = SEL_ROW

    shared = {
        "wq": wq, "wk": wk, "wv": wv,
        "wp": Wp.astype(np.float32),
        "wr": wr,
        "w1q": w1q, "w2q": w2q,
        "selc": selc,
    }

    in_maps = []
    for c in range(NCORES):
        b, half = divmod(c, 2)
        perm = np.r_[half * S:(half + 1) * S, (1 - half) * S:(2 - half) * S]
        xbt = np.ascontiguousarray(x[b][perm].T)  # [C, T], my tokens first
        kvb = np.zeros((128, 8), np.float32)
        if half == 0:
            kvb[:, 4:] = NEG
        m = dict(shared)
        m["xbt"] = xbt
        m["kvb"] = kvb
        in_maps.append(m)
    return in_maps


def gather_outputs(results):
    out = np.empty((B, T, C), np.float32)
    for c in range(NCORES):
        b, half = divmod(c, 2)
        out[b, half * S:(half + 1) * S, :] = results[c]["out"].T
    return out


def kernel(**inputs):
    from concourse.bass_utils import run_bass_kernel_spmd

    if "nc" not in _CACHE:
        _CACHE["nc"] = build_program()
    nc = _CACHE["nc"]
    in_maps = prep_inputs(inputs)
    res = run_bass_kernel_spmd(nc, in_maps, list(range(NCORES)))
    return gather_outputs(res.results)

